# revision 13
# baseline (speedup 1.0000x reference)
"""Toeplitz bias kernel for trn2 (8 NeuronCores).

bias[h, j, i] = exp(w_[h] - offset[h])[2*L-2 + j - i]   with L = 2048.

Let p = exp(w_[h] - offset[h]) (length S = 2*L-1 = 4095) and
q[k] = p[S-1-k] (the reverse). Then bias[h, j, i] = q[L-1 - j + i].

Per-head construction on device (no negative strides anywhere):
  - q = exp(reverse(w_) - offset), with the reversal done on host as a pure
    layout transform of the input.
  - staircase tile qrep[t, c] = q[c - t] for t in [0, 128), built by 7
    log-doubling SBUF->SBUF DMA copies:
        qrep[d:2d, d:S] = qrep[0:d, 0:S-d]
    (row t's columns c < t are garbage but never read).
  - every 128-row output block is then a plain slice store:
        bias[j0 + t, i] = q[L-1 - j0 - t + i] = qrep[t, (L-1-j0) + i]
        => out[j0:j0+128, :] = qrep[:, L-1-j0 : L-1-j0+L]
    which is a contiguous 1MB DMA per block.

Heads are sharded 2 per core across the 8 cores; each core writes its own
[2, L, L] output and the host concatenates.
"""

import numpy as np

H = 16
L = 2048
S = 2 * L - 1  # 4095
N_CORES = 8
HPC = H // N_CORES  # heads per core
NBLK = L // 128  # 16 row blocks per head

_cached_nc = None


def _build_nc(inner_reps=1, nop=False, store_eng="hwdge"):
    import concourse.bacc as bacc
    import concourse.mybir as mybir
    import concourse.tile as tile

    nc = bacc.Bacc("TRN2", target_bir_lowering=False)
    f32 = mybir.dt.float32
    # win[:, 0:S] = reversed w_ rows; win[:, S] = -offset  (host-packed so the
    # whole preamble is one DMA -> one activation)
    win = nc.dram_tensor("win", [HPC, S + 1], f32, kind="ExternalInput")
    out = nc.dram_tensor("out", [HPC, L, L], f32, kind="ExternalOutput")

    with tile.TileContext(nc) as tc:
        with tc.tile_pool(name="p", bufs=2 if inner_reps > 1 else 1) as pool:
            if nop:
                t = pool.tile([1, 16], f32, tag="t")
                nc.sync.dma_start(t[:, :], win[0:1, 0:16])
                nc.sync.dma_start(out[0, 0:1, 0:16], t[:, :])
            for _rep in range(0 if nop else inner_reps):
                wt = pool.tile([HPC, S + 1], f32, tag="wt")
                qt = pool.tile([HPC, S], f32, tag="qt")
                nc.sync.dma_start(wt[:, :], win[:, :])
                # qt = exp(w_rev + (-offset)); bias: per-partition scalar AP
                nc.scalar.activation(
                    qt[:, :],
                    wt[:, 0:S],
                    mybir.ActivationFunctionType.Exp,
                    bias=wt[:, S : S + 1],
                )

                qreps = []
                for h in range(HPC):
                    eng = nc.sync if h % 2 == 0 else nc.scalar
                    qr = pool.tile([128, S], f32, tag=f"qrep{h}")
                    qreps.append(qr)
                    eng.dma_start(qr[0:1, :], qt[h : h + 1, :])
                    for m in range(7):
                        d = 1 << m
                        eng.dma_start(qr[d : 2 * d, d:S], qr[0:d, 0 : S - d])

                for b in range(NBLK):
                    j0 = 128 * b
                    c0 = L - 1 - j0
                    for h in range(HPC):
                        if store_eng == "hwdge":
                            eng = nc.sync if h % 2 == 0 else nc.scalar
                        elif store_eng == "gpsimd":
                            eng = nc.gpsimd
                        elif store_eng == "mixed3":
                            eng = (nc.gpsimd, nc.sync, nc.scalar)[(2 * b + h) % 3]
                        else:
                            raise ValueError(store_eng)
                        eng.dma_start(
                            out[h, j0 : j0 + 128, :], qreps[h][:, c0 : c0 + L]
                        )
    nc.compile()
    return nc


def _get_nc():
    global _cached_nc
    if _cached_nc is None:
        _cached_nc = _build_nc()
    return _cached_nc


def _make_in_maps(w_, offset):
    w_ = np.asarray(w_, dtype=np.float32)
    offset = np.asarray(offset, dtype=np.float32)
    win = np.empty((H, S + 1), dtype=np.float32)
    win[:, 0:S] = w_[:, ::-1]
    win[:, S] = -offset
    in_maps = []
    for c in range(N_CORES):
        sl = slice(c * HPC, (c + 1) * HPC)
        in_maps.append({"win": np.ascontiguousarray(win[sl])})
    return in_maps


def run(w_, offset, trace=False, **trace_kw):
    from concourse.bass_utils import run_bass_kernel_spmd

    nc = _get_nc()
    in_maps = _make_in_maps(w_, offset)
    res = run_bass_kernel_spmd(
        nc, in_maps, list(range(N_CORES)), trace=trace, **trace_kw
    )
    parts = [np.asarray(r["out"]) for r in res.results]
    full = np.concatenate(parts, axis=0)  # [H, L, L]
    return full, res


def kernel(w_, offset, seq_len=None, **_ignored):
    full, _ = run(w_, offset, trace=False)
    return full


def bench(w_, offset, reps=8, inner_reps=1, nop=False, store_eng="hwdge"):
    """Estimate per-iteration device execution time by pipelining `reps`
    executions of the sharded jit (mirrors run_bass_via_pjrt's multi-core
    path). Returns (est_ns_per_iter, full_output)."""
    import time

    import jax
    from jax.sharding import Mesh, PartitionSpec
    from jax.experimental.shard_map import shard_map

    import concourse.mybir as mybir
    from concourse import bass2jax

    bass2jax.install_neuronx_cc_hook()
    nc = _build_nc(inner_reps=inner_reps, nop=nop, store_eng=store_eng)
    in_maps = _make_in_maps(w_, offset)
    n_cores = N_CORES

    partition_name = nc.partition_id_tensor.name if nc.partition_id_tensor else None
    in_names, out_names, out_avals, zero_outs = [], [], [], []
    for alloc in nc.m.functions[0].allocations:
        if not isinstance(alloc, mybir.MemoryLocationSet):
            continue
        name = alloc.memorylocations[0].name
        if alloc.kind == "ExternalInput":
            if name != partition_name:
                in_names.append(name)
        elif alloc.kind == "ExternalOutput":
            shape = tuple(alloc.tensor_shape)
            dtype = mybir.dt.np(alloc.dtype)
            out_names.append(name)
            out_avals.append(jax.core.ShapedArray(shape, dtype))
            zero_outs.append(np.zeros(shape, dtype))
    n_params = len(in_names)
    n_outs = len(out_avals)
    in_names_all = in_names + out_names
    if partition_name is not None:
        in_names_all.append(partition_name)

    def _body(*args):
        operands = list(args)
        if partition_name is not None:
            operands.append(bass2jax.partition_id_tensor())
        outs = bass2jax._bass_exec_p.bind(
            *operands,
            out_avals=tuple(out_avals),
            in_names=tuple(in_names_all),
            out_names=tuple(out_names),
            lowering_input_output_aliases=(),
            sim_require_finite=True,
            sim_require_nnan=True,
            nc=nc,
        )
        return tuple(outs)

    devices = jax.devices()[:n_cores]
    mesh = Mesh(np.asarray(devices), ("core",))
    in_specs = (PartitionSpec("core"),) * (n_params + n_outs)
    out_specs = (PartitionSpec("core"),) * n_outs
    donate = tuple(range(n_params, n_params + n_outs))
    sharded = jax.jit(
        shard_map(
            _body, mesh=mesh, in_specs=in_specs, out_specs=out_specs, check_rep=False
        ),
        donate_argnums=donate,
        keep_unused=True,
    )

    per_core = [[np.asarray(m[name]) for name in in_names] for m in in_maps]
    concat_in = [
        np.concatenate([per_core[c][i] for c in range(n_cores)], axis=0)
        for i in range(n_params)
    ]
    sharding = jax.sharding.NamedSharding(mesh, PartitionSpec("core"))
    dev_in = [jax.device_put(a, sharding) for a in concat_in]
    zshapes = [(n_cores * z.shape[0], *z.shape[1:]) for z in zero_outs]

    def fresh_zeros():
        return [
            jax.device_put(np.zeros(s, z.dtype), sharding)
            for s, z in zip(zshapes, zero_outs)
        ]

    # warmup (compiles)
    warm = sharded(*dev_in, *fresh_zeros())
    out_np = [np.asarray(o) for o in warm]
    del warm

    staged = [fresh_zeros() for _ in range(reps)]
    for zs in staged:
        jax.block_until_ready(zs)

    t0 = time.perf_counter()
    last = None
    for zs in staged:
        last = sharded(*dev_in, *zs)
    jax.block_until_ready(last)
    t1 = time.perf_counter()
    total_ns = (t1 - t0) * 1e9
    est = total_ns / reps

    full = np.concatenate(
        [
            out_np[0].reshape(n_cores, HPC, L, L)[c]
            for c in range(n_cores)
        ],
        axis=0,
    )
    return est, full
